# revision 15
# baseline (speedup 1.0000x reference)
"""Trainium2 Bass kernel for block-diagonal sparse attention (8 NeuronCores SPMD).

Problem: nn_AttentionHead (N=4096, DIM_IN=512, DQ=DK=128, 16 graphs of 256 nodes).
  q = x@Wq.T+bq; k = x@Wk.T+bk; v = x@Wv.T+bv
  a = where(block, qk/sqrt(dq), 0) + b + c; masked-softmax over block-diagonal
  out = (softmax(a)*keep) @ v

Key structural facts exploited:
  - Only the 16 diagonal 256x256 tiles of b/c/sparse_mask matter; the host
    slices them, combines bcm = b+c (masked entries -> -200 so exp gives 0),
    casts to bf16. HBM traffic is ~1.3MB/core instead of ~200MB.
  - Graphs are independent -> 2 graphs per core across 8 cores, zero cross-core
    communication (weights replicated).
  - bcm is added into the score PSUM by the PE itself via an identity-matmul
    accumulated onto the qk matmul, so the only post-processing is a single
    exp per graph straight out of the (single-bank) PSUM tile.
  - The denominator is obtained free by appending a ones-column to v in the PV
    matmul; the division happens on the HOST (outputs leave the chip
    unnormalized as [num | den] rows in bf16).
  - 1/sqrt(dq) is folded into Wq host-side; everything is pre-cast to bf16.
  - The PE HAM clock-gate unthrottles 1.2->2.4GHz only after ~4us of gapless
    matmul activity, so dummy warmup matmuls start as early as possible (warm
    tile memset on gpsimd, whose preamble ends first) and bridge the entire
    input-DMA phase; the real matmuls then all run at 2.4GHz.
  - bacc hoists ACT_TABLE_LOAD (1.3us) to the top of the Scalar engine stream,
    so the Scalar ring only carries late-needed tensors (wv/identity, bcm);
    x rides the sync ring.
"""

import math

import numpy as np
import ml_dtypes

import concourse.bass as bass
import concourse.mybir as mybir
import concourse.tile as tile
from concourse import bacc
from concourse.bass_utils import run_bass_kernel_spmd

# -------- problem constants (hardcoded per spec) --------
N = 4096
DIN = 512
DQ = 128           # == DK
NG = 16            # number of graphs
G = N // NG        # 256 nodes per graph
NCORES = 8
RPC = N // NCORES  # 512 rows per core
GPC = NG // NCORES  # 2 graphs per core
NT = RPC // 128    # 4 row-tiles of 128 per core
KO = DIN // 128    # 4 contraction tiles for the projections
VA = DQ + 1        # v augmented with a ones column (denominator trick)
SCALE = 1.0 / math.sqrt(DQ)
NEG = -200.0       # masked-entry sentinel; exp(-200 + |qk|max) == 0 in bf16
NWARM = 11         # PE HAM warmup matmuls (~4.7us cold from ~7us)

F32 = mybir.dt.float32
BF16 = mybir.dt.bfloat16

ACT = mybir.ActivationFunctionType
ALU = mybir.AluOpType

BF = ml_dtypes.bfloat16

_CACHE: dict = {}


def build_nc() -> bass.Bass:
    """Build the per-core Bass graph (identical on all 8 cores)."""
    nc = bacc.Bacc(
        "TRN2",
        target_bir_lowering=False,
        debug=False,
        enable_asserts=False,
        num_devices=NCORES,
    )
    xT_d = nc.dram_tensor("xh", [128, KO, RPC], BF16, kind="ExternalInput").ap()
    wqk_d = nc.dram_tensor("wqk", [128, 2, KO, DQ], BF16, kind="ExternalInput").ap()
    # wv with the 128x128 identity appended on the free axis
    wvi_d = nc.dram_tensor("wvi", [128, KO * DQ + 128], BF16, kind="ExternalInput").ap()
    bia_d = nc.dram_tensor("bias", [DQ, 3], F32, kind="ExternalInput").ap()
    biar_d = nc.dram_tensor("biasr", [1, 3, DQ], BF16, kind="ExternalInput").ap()
    bc_d = nc.dram_tensor("bch", [128, GPC, 2 * G], BF16, kind="ExternalInput").ap()
    out_d = nc.dram_tensor("out", [128, NT, VA], BF16, kind="ExternalOutput").ap()

    with tile.TileContext(nc) as tc:
        with (
            tc.tile_pool(name="const", bufs=1) as cpool,
            tc.tile_pool(name="eq", bufs=2) as epool,
            tc.tile_pool(name="ps_proj", bufs=2, space="PSUM") as pp,
            tc.tile_pool(name="ps_v", bufs=2, space="PSUM") as pvp,
            tc.tile_pool(name="ps_s", bufs=2, space="PSUM") as ps,
            tc.tile_pool(name="ps_o", bufs=2, space="PSUM") as po,
        ):
            # warm tile on gpsimd (its preamble finishes first) so the PE
            # warmup starts as early as possible; only the lhsT columns need
            # defined data -- the rhs may read stale SBUF, the results are
            # never consumed
            warm = cpool.tile([128, RPC], BF16)
            nc.gpsimd.memset(warm[:, 0:128], 1.0)

            # ---- input DMAs. Scalar (=ACT) engine's stream begins with the
            # hoisted 1.3us ACT_TABLE_LOAD, so it only carries late-needed
            # tensors; x + q/k weights ride the sync ring.
            wqk = cpool.tile([128, 2, KO, DQ], BF16)
            nc.sync.dma_start(wqk[:], wqk_d)
            xT = cpool.tile([128, KO, RPC], BF16)  # [din%128, din//128, r]
            nc.scalar.dma_start(xT[:, 2:4, :], xT_d[:, 2:4, :])
            nc.sync.dma_start(xT[:, 0:2, :], xT_d[:, 0:2, :])
            wvi = cpool.tile([128, KO * DQ + 128], BF16)
            nc.scalar.dma_start(wvi[:], wvi_d)
            bia = cpool.tile([128, 3], F32)  # [d, qkv]; q column pre-scaled
            nc.sync.dma_start(bia[:], bia_d)
            bc = cpool.tile([128, GPC, 2 * G], BF16)  # bcm blocks, transposed
            nc.scalar.dma_start(bc[:], bc_d)
            biar = cpool.tile([1, 3, DQ], BF16)  # row layout for the v bias
            nc.sync.dma_start(biar[:], biar_d)
            idn = wvi[:, KO * DQ:KO * DQ + 128]

            ones_b = cpool.tile([1, 128], BF16)  # rank-1 bias lhsT
            nc.vector.memset(ones_b[:], 1.0)
            vna = cpool.tile([128, NT, VA], BF16)  # [j%128, j//128, d | 1]
            nc.vector.memset(vna[:, :, DQ:VA], 1.0)

            # ---- PE HAM warmup: continuous dummy matmuls across the DMA
            # phase; the K=8/8 unthrottle fires during warmup, so the real
            # matmuls below all run at 2.4GHz
            for _ in range(NWARM):
                wp = pp.tile([128, RPC], F32, tag="proj")
                nc.tensor.matmul(
                    wp[:], lhsT=warm[:, 0:128], rhs=warm[:],
                    start=True, stop=True,
                )

            # ---- q/k projections, transposed: pT[d, r] = (x @ W_s.T).T ----
            def proj(s):
                p = pp.tile([128, RPC], F32, tag="proj")
                for ko in range(KO):
                    nc.tensor.matmul(
                        p[:],
                        lhsT=wqk[:, s, ko, :],
                        rhs=xT[:, ko, :],
                        start=(ko == 0),
                        stop=(ko == KO - 1),
                    )
                return p

            # evacuate in graph-halves; q on DVE, k on ACT in parallel
            pq = proj(0)
            qT = cpool.tile([128, RPC], BF16)
            nc.vector.tensor_scalar_add(qT[:, 0:G], pq[:, 0:G], bia[:, 0:1])
            nc.vector.tensor_scalar_add(qT[:, G:RPC], pq[:, G:RPC], bia[:, 0:1])
            pk = proj(1)
            kT = cpool.tile([128, RPC], BF16)
            nc.scalar.activation(kT[:, 0:G], pk[:, 0:G], ACT.Identity, bias=bia[:, 1:2])
            nc.scalar.activation(kT[:, G:RPC], pk[:, G:RPC], ACT.Identity, bias=bia[:, 1:2])

            def proj_v(jt):
                pv = pvp.tile([128, DQ], F32, tag="vn")
                first = None
                for ko in range(KO):
                    mi = nc.tensor.matmul(
                        pv[:],
                        lhsT=xT[:, ko, jt * 128:(jt + 1) * 128],
                        rhs=wvi[:, ko * DQ:(ko + 1) * DQ],
                        start=(ko == 0), stop=False,
                    )
                    if first is None:
                        first = mi
                nc.tensor.matmul(
                    pv[:], lhsT=ones_b[:], rhs=biar[:, 2, :],
                    start=False, stop=True,
                )
                nc.vector.tensor_copy(out=vna[:, jt, 0:DQ], in_=pv[:])
                return first

            eqs = [None, None]

            def scores_graph(g):
                """qk scores + bcm via identity-matmul, one exp per graph."""
                spg = ps.tile([128, 2 * G], F32, tag="s")  # 1 bank, both j-blocks
                for jb in range(2):
                    t = 2 * g + jb
                    nc.tensor.matmul(
                        spg[:, jb * G:(jb + 1) * G],
                        lhsT=kT[:, t * 128:(t + 1) * 128],
                        rhs=qT[:, g * G:(g + 1) * G],
                        start=(jb == 0), stop=False,
                        skip_group_check=True,
                    )
                last = None
                for jb in range(2):
                    last = nc.tensor.matmul(
                        spg[:, jb * G:(jb + 1) * G],
                        lhsT=idn,
                        rhs=bc[:, g, jb * G:(jb + 1) * G],
                        start=False, stop=(jb == 1),
                        skip_group_check=True,
                    )
                eq = epool.tile([128, 2 * G], BF16, tag="eq")
                nc.scalar.activation(eq[:], spg[:], ACT.Exp)
                eqs[g] = eq
                return last

            out_sb = cpool.tile([128, NT, VA], BF16)

            def pv_graph(g):
                """PV matmuls (+denominator column) for both row-tiles of a
                graph into ONE PSUM bank, single evacuation, one store."""
                op = po.tile([128, 2, VA], F32, tag="o")
                for rb in range(2):
                    for jb in range(2):
                        nc.tensor.matmul(
                            op[:, rb, :],
                            lhsT=eqs[g][:, jb * G + rb * 128: jb * G + rb * 128 + 128],
                            rhs=vna[:, 2 * g + jb, :],
                            start=(rb == 0 and jb == 0), stop=(rb == 1 and jb == 1),
                            skip_group_check=True,
                        )
                nc.vector.tensor_copy(
                    out=out_sb[:, 2 * g:2 * g + 2, :], in_=op[:]
                )
                nc.sync.dma_start(
                    out_d[:, 2 * g:2 * g + 2, :], out_sb[:, 2 * g:2 * g + 2, :]
                )

            # interleave the score groups between v-tiles and pin the order
            # with explicit deps (the scheduler otherwise runs all v-tiles
            # first, pushing the serial ACT exp chain ~1us later)
            proj_v(0)
            proj_v(1)
            sc0 = scores_graph(0)
            v2 = proj_v(2)
            sc1 = scores_graph(1)
            v3 = proj_v(3)
            tile.add_dep_helper(
                v2.ins, sc0.ins, sync=False, reason="run scores g0 before v2"
            )
            tile.add_dep_helper(
                v3.ins, sc1.ins, sync=False, reason="run scores g1 before v3"
            )
            pv_graph(0)
            pv_graph(1)
    nc.compile()
    return nc


def get_nc() -> bass.Bass:
    if "nc" not in _CACHE:
        _CACHE["nc"] = build_nc()
    return _CACHE["nc"]


def make_in_maps(x, b, c, ptr, sparse_mask, Wq, bq, Wk, bk, Wv, bv):
    """Host-side sharding: slice the block-diagonal, combine b+c with the mask
    sentinel, cast everything to bf16, transpose to partition-major layouts."""
    x = np.asarray(x, dtype=np.float32)
    b = np.asarray(b, dtype=np.float32)
    c = np.asarray(c, dtype=np.float32)
    ptr = np.asarray(ptr)
    mask = np.asarray(sparse_mask) != 0
    # fold 1/sqrt(dq) into Wq/bq so scores come out pre-scaled
    wq3 = (np.asarray(Wq).T * SCALE).astype(np.float32)
    wk3 = np.asarray(Wk).T.astype(np.float32)
    wv3 = np.asarray(Wv).T.astype(np.float32)  # each [DIN, DQ]
    bias = np.ascontiguousarray(
        np.stack(
            [np.asarray(bq) * SCALE, np.asarray(bk), np.asarray(bv)], axis=1
        )
    ).astype(np.float32)  # [DQ, 3]
    biasr = np.ascontiguousarray(
        np.stack([np.asarray(bq), np.asarray(bk), np.asarray(bv)], axis=0)[None]
    ).astype(BF)  # [1, 3, DQ]

    assert np.array_equal(
        np.asarray(ptr).ravel(), np.arange(NG + 1) * G
    ), "kernel compiled for uniform 256-node graphs"

    def wshape(w3):  # [128, KO, DQ], partition-major over DIN
        return np.ascontiguousarray(
            w3.reshape(KO, 128, DQ).transpose(1, 0, 2)
        ).astype(BF)

    wqkh = np.ascontiguousarray(
        np.stack([wshape(wq3), wshape(wk3)], axis=1)
    )  # [128, 2, KO, DQ]
    wvih = np.ascontiguousarray(
        np.concatenate(
            [wshape(wv3).reshape(128, KO * DQ), np.eye(128, dtype=BF)], axis=1
        )
    )  # [128, KO*DQ + 128]

    in_maps = []
    for i in range(NCORES):
        lo = i * RPC
        xT = x[lo:lo + RPC].T  # [DIN, RPC]
        xh = np.ascontiguousarray(
            xT.reshape(KO, 128, RPC).transpose(1, 0, 2)
        ).astype(BF)  # [128, KO, RPC]
        bch = np.empty((128, GPC, 2 * G), dtype=np.float32)
        for gl in range(GPC):
            blk = slice(lo + gl * G, lo + (gl + 1) * G)
            m = np.where(mask[blk, blk], b[blk, blk] + c[blk, blk], NEG).T
            # bch[p, gl, jb*G + r] = m[jb*128+p, r]
            bch[:, gl, :] = m.reshape(2, 128, G).transpose(1, 0, 2).reshape(128, 2 * G)
        bch = np.ascontiguousarray(bch).astype(BF)
        in_maps.append(
            {"xh": xh, "wqk": wqkh, "wvi": wvih, "bias": bias, "biasr": biasr,
             "bch": bch}
        )
    return in_maps


def run(inputs: dict, trace: bool = False):
    """Run on all 8 cores; returns (full_output, BassKernelResults)."""
    nc = get_nc()
    in_maps = make_in_maps(**inputs)
    res = run_bass_kernel_spmd(
        nc, in_maps, core_ids=list(range(NCORES)), trace=trace
    )
    outs = []
    for r in res.results:
        o = np.asarray(r["out"]).astype(np.float32)  # [128, NT, VA]
        o = o[:, :, 0:DQ] / o[:, :, DQ:VA]  # host-side softmax normalization
        outs.append(o.transpose(1, 0, 2).reshape(RPC, DQ))
    out = np.concatenate(outs, axis=0)
    return out, res


def kernel(**inputs) -> np.ndarray:
    out, _ = run(inputs, trace=False)
    return out


# revision 16
# speedup vs baseline: 1.0561x; 1.0561x over previous
"""Trainium2 Bass kernel for block-diagonal sparse attention (8 NeuronCores SPMD).

Problem: nn_AttentionHead (N=4096, DIM_IN=512, DQ=DK=128, 16 graphs of 256 nodes).
  q = x@Wq.T+bq; k = x@Wk.T+bk; v = x@Wv.T+bv
  a = where(block, qk/sqrt(dq), 0) + b + c; masked-softmax over block-diagonal
  out = (softmax(a)*keep) @ v

Key structural facts exploited:
  - Only the 16 diagonal 256x256 tiles of b/c/sparse_mask matter; the host
    slices them, combines bcm = b+c (masked entries -> -200 so exp gives 0),
    casts to bf16. HBM traffic is ~1.3MB/core instead of ~200MB.
  - Graphs are independent -> 2 graphs per core across 8 cores, zero cross-core
    communication (weights replicated).
  - bcm is added into the score PSUM by the PE itself via an identity-matmul
    accumulated onto the qk matmul, so the only post-processing is a single
    exp per graph straight out of the (single-bank) PSUM tile.
  - The denominator is obtained free by appending a ones-column to v in the PV
    matmul; the division happens on the HOST (outputs leave the chip
    unnormalized as [num | den] rows in bf16).
  - 1/sqrt(dq) is folded into Wq host-side; everything is pre-cast to bf16.
  - The PE HAM clock-gate unthrottles 1.2->2.4GHz only after ~4us of gapless
    matmul activity, so dummy warmup matmuls bridge the entire input-DMA
    phase; the real matmuls then all run at 2.4GHz.
  - q/k are projected in graph-column halves with per-half PSUM evacuation, so
    graph 0's score/exp chain starts while graph 1 is still projecting.
  - All weights ride in ONE wall transfer (per-transfer ramp/bubbles measurably
    fragment DMA bandwidth); bacc hoists ACT_TABLE_LOAD (1.3us) to the top of
    the Scalar stream, so the Scalar ring carries only x and bcm.
"""

import math

import numpy as np
import ml_dtypes

import concourse.bass as bass
import concourse.mybir as mybir
import concourse.tile as tile
from concourse import bacc
from concourse.bass_utils import run_bass_kernel_spmd

# -------- problem constants (hardcoded per spec) --------
N = 4096
DIN = 512
DQ = 128           # == DK
NG = 16            # number of graphs
G = N // NG        # 256 nodes per graph
NCORES = 8
RPC = N // NCORES  # 512 rows per core
GPC = NG // NCORES  # 2 graphs per core
NT = RPC // 128    # 4 row-tiles of 128 per core
KO = DIN // 128    # 4 contraction tiles for the projections
VA = DQ + 1        # v augmented with a ones column (denominator trick)
SCALE = 1.0 / math.sqrt(DQ)
NEG = -200.0       # masked-entry sentinel; exp(-200 + |qk|max) == 0 in bf16
NWARM = 12         # PE HAM warmup matmuls

F32 = mybir.dt.float32
BF16 = mybir.dt.bfloat16

ACT = mybir.ActivationFunctionType
ALU = mybir.AluOpType

BF = ml_dtypes.bfloat16

WALL = (2 * KO + KO) * DQ + 128  # wq | wk | wv | identity columns

_CACHE: dict = {}


def build_nc() -> bass.Bass:
    """Build the per-core Bass graph (identical on all 8 cores)."""
    nc = bacc.Bacc(
        "TRN2",
        target_bir_lowering=False,
        debug=False,
        enable_asserts=False,
        num_devices=NCORES,
    )
    xT_d = nc.dram_tensor("xh", [128, KO, RPC], BF16, kind="ExternalInput").ap()
    wall_d = nc.dram_tensor("wall", [128, WALL], BF16, kind="ExternalInput").ap()
    bia_d = nc.dram_tensor("bias", [DQ, 3], F32, kind="ExternalInput").ap()
    biar_d = nc.dram_tensor("biasr", [1, 3, DQ], BF16, kind="ExternalInput").ap()
    bc_d = nc.dram_tensor("bch", [128, GPC, 2 * G], BF16, kind="ExternalInput").ap()
    out_d = nc.dram_tensor("out", [128, NT, VA], BF16, kind="ExternalOutput").ap()

    with tile.TileContext(nc) as tc:
        with (
            tc.tile_pool(name="const", bufs=1) as cpool,
            tc.tile_pool(name="eq", bufs=2) as epool,
            tc.tile_pool(name="ps_proj", bufs=2, space="PSUM") as pp,
            tc.tile_pool(name="ps_v", bufs=2, space="PSUM") as pvp,
            tc.tile_pool(name="ps_s", bufs=2, space="PSUM") as ps,
            tc.tile_pool(name="ps_o", bufs=2, space="PSUM") as po,
        ):
            # warm tile on gpsimd (its preamble finishes first) so the PE
            # warmup starts as early as possible; only the lhsT columns need
            # defined data -- the rhs may read stale SBUF
            warm = cpool.tile([128, RPC], BF16)
            nc.gpsimd.memset(warm[:, 0:128], 1.0)

            # ---- input DMAs ----
            wall = cpool.tile([128, WALL], BF16)
            nc.sync.dma_start(wall[:], wall_d)
            xT = cpool.tile([128, KO, RPC], BF16)  # [din%128, din//128, r]
            nc.scalar.dma_start(xT[:], xT_d)
            bia = cpool.tile([128, 3], F32)  # [d, qkv]; q column pre-scaled
            nc.sync.dma_start(bia[:], bia_d)
            bc = cpool.tile([128, GPC, 2 * G], BF16)  # bcm blocks, transposed
            nc.scalar.dma_start(bc[:], bc_d)
            biar = cpool.tile([1, 3, DQ], BF16)  # row layout for the v bias
            nc.sync.dma_start(biar[:], biar_d)

            def wsl(s, ko):  # weight slice for projection s, contraction ko
                o = (s * KO + ko) * DQ
                return wall[:, o:o + DQ]

            idn = wall[:, 3 * KO * DQ:3 * KO * DQ + 128]

            ones_b = cpool.tile([1, 128], BF16)  # rank-1 bias lhsT
            nc.vector.memset(ones_b[:], 1.0)
            vna = cpool.tile([128, NT, VA], BF16)  # [j%128, j//128, d | 1]
            nc.vector.memset(vna[:, :, DQ:VA], 1.0)

            # ---- PE HAM warmup ----
            for _ in range(NWARM):
                wp = pp.tile([128, RPC], F32, tag="proj")
                nc.tensor.matmul(
                    wp[:], lhsT=warm[:, 0:128], rhs=warm[:],
                    start=True, stop=True,
                )

            qT = cpool.tile([128, RPC], BF16)
            kT = cpool.tile([128, RPC], BF16)

            # ---- q/k projections in graph-column halves; per-half evac so
            # graph 0's scores start while graph 1 still projects.
            pq = pp.tile([128, RPC], F32, tag="proj")
            pk = pp.tile([128, RPC], F32, tag="proj")

            def proj_half(s, p, h):
                hs = slice(h * G, (h + 1) * G)
                for ko in range(KO):
                    nc.tensor.matmul(
                        p[:, hs], lhsT=wsl(s, ko), rhs=xT[:, ko, hs],
                        start=(ko == 0), stop=(ko == KO - 1),
                        skip_group_check=True,
                    )
                if s == 0:
                    nc.vector.tensor_scalar_add(qT[:, hs], p[:, hs], bia[:, 0:1])
                else:
                    nc.scalar.activation(
                        kT[:, hs], p[:, hs], ACT.Identity, bias=bia[:, 1:2]
                    )

            def proj_v(jt):
                pv = pvp.tile([128, DQ], F32, tag="vn")
                first = None
                for ko in range(KO):
                    mi = nc.tensor.matmul(
                        pv[:],
                        lhsT=xT[:, ko, jt * 128:(jt + 1) * 128],
                        rhs=wsl(2, ko),
                        start=(ko == 0), stop=False,
                    )
                    if first is None:
                        first = mi
                nc.tensor.matmul(
                    pv[:], lhsT=ones_b[:], rhs=biar[:, 2, :],
                    start=False, stop=True,
                )
                nc.vector.tensor_copy(out=vna[:, jt, 0:DQ], in_=pv[:])
                return first

            eqs = [None, None]

            def scores_graph(g):
                """qk scores + bcm via identity-matmul, one exp per graph."""
                spg = ps.tile([128, 2 * G], F32, tag="s")  # 1 bank, both j-blocks
                for jb in range(2):
                    t = 2 * g + jb
                    nc.tensor.matmul(
                        spg[:, jb * G:(jb + 1) * G],
                        lhsT=kT[:, t * 128:(t + 1) * 128],
                        rhs=qT[:, g * G:(g + 1) * G],
                        start=(jb == 0), stop=False,
                        skip_group_check=True,
                    )
                last = None
                for jb in range(2):
                    last = nc.tensor.matmul(
                        spg[:, jb * G:(jb + 1) * G],
                        lhsT=idn,
                        rhs=bc[:, g, jb * G:(jb + 1) * G],
                        start=False, stop=(jb == 1),
                        skip_group_check=True,
                    )
                eq = epool.tile([128, 2 * G], BF16, tag="eq")
                nc.scalar.activation(eq[:], spg[:], ACT.Exp)
                eqs[g] = eq
                return last

            out_sb = cpool.tile([128, NT, VA], BF16)

            def pv_graph(g):
                """PV matmuls (+denominator column) for both row-tiles of a
                graph into ONE PSUM bank, single evacuation, one store."""
                op = po.tile([128, 2, VA], F32, tag="o")
                for rb in range(2):
                    for jb in range(2):
                        nc.tensor.matmul(
                            op[:, rb, :],
                            lhsT=eqs[g][:, jb * G + rb * 128: jb * G + rb * 128 + 128],
                            rhs=vna[:, 2 * g + jb, :],
                            start=(rb == 0 and jb == 0), stop=(rb == 1 and jb == 1),
                            skip_group_check=True,
                        )
                nc.vector.tensor_copy(
                    out=out_sb[:, 2 * g:2 * g + 2, :], in_=op[:]
                )
                nc.sync.dma_start(
                    out_d[:, 2 * g:2 * g + 2, :], out_sb[:, 2 * g:2 * g + 2, :]
                )

            proj_half(0, pq, 0)   # q graph 0
            proj_half(1, pk, 0)   # k graph 0
            proj_half(0, pq, 1)   # q graph 1
            proj_half(1, pk, 1)   # k graph 1
            proj_v(0)
            proj_v(1)
            sc0 = scores_graph(0)
            v2 = proj_v(2)
            sc1 = scores_graph(1)
            v3 = proj_v(3)
            tile.add_dep_helper(
                v2.ins, sc0.ins, sync=False, reason="run scores g0 before v2"
            )
            tile.add_dep_helper(
                v3.ins, sc1.ins, sync=False, reason="run scores g1 before v3"
            )
            pv_graph(0)
            pv_graph(1)
    nc.compile()
    return nc


def get_nc() -> bass.Bass:
    if "nc" not in _CACHE:
        _CACHE["nc"] = build_nc()
    return _CACHE["nc"]


def make_in_maps(x, b, c, ptr, sparse_mask, Wq, bq, Wk, bk, Wv, bv):
    """Host-side sharding: slice the block-diagonal, combine b+c with the mask
    sentinel, cast everything to bf16, transpose to partition-major layouts."""
    x = np.asarray(x, dtype=np.float32)
    b = np.asarray(b, dtype=np.float32)
    c = np.asarray(c, dtype=np.float32)
    ptr = np.asarray(ptr)
    mask = np.asarray(sparse_mask) != 0
    # fold 1/sqrt(dq) into Wq/bq so scores come out pre-scaled
    wq3 = (np.asarray(Wq).T * SCALE).astype(np.float32)
    wk3 = np.asarray(Wk).T.astype(np.float32)
    wv3 = np.asarray(Wv).T.astype(np.float32)  # each [DIN, DQ]
    bias = np.ascontiguousarray(
        np.stack(
            [np.asarray(bq) * SCALE, np.asarray(bk), np.asarray(bv)], axis=1
        )
    ).astype(np.float32)  # [DQ, 3]
    biasr = np.ascontiguousarray(
        np.stack([np.asarray(bq), np.asarray(bk), np.asarray(bv)], axis=0)[None]
    ).astype(BF)  # [1, 3, DQ]

    assert np.array_equal(
        np.asarray(ptr).ravel(), np.arange(NG + 1) * G
    ), "kernel compiled for uniform 256-node graphs"

    def wshape(w3):  # [128, KO*DQ], partition-major over DIN
        return np.ascontiguousarray(
            w3.reshape(KO, 128, DQ).transpose(1, 0, 2)
        ).astype(BF).reshape(128, KO * DQ)

    wallh = np.ascontiguousarray(
        np.concatenate(
            [wshape(wq3), wshape(wk3), wshape(wv3), np.eye(128, dtype=BF)],
            axis=1,
        )
    )  # [128, WALL]

    in_maps = []
    for i in range(NCORES):
        lo = i * RPC
        xT = x[lo:lo + RPC].T  # [DIN, RPC]
        xh = np.ascontiguousarray(
            xT.reshape(KO, 128, RPC).transpose(1, 0, 2)
        ).astype(BF)  # [128, KO, RPC]
        bch = np.empty((128, GPC, 2 * G), dtype=np.float32)
        for gl in range(GPC):
            blk = slice(lo + gl * G, lo + (gl + 1) * G)
            m = np.where(mask[blk, blk], b[blk, blk] + c[blk, blk], NEG).T
            # bch[p, gl, jb*G + r] = m[jb*128+p, r]
            bch[:, gl, :] = m.reshape(2, 128, G).transpose(1, 0, 2).reshape(128, 2 * G)
        bch = np.ascontiguousarray(bch).astype(BF)
        in_maps.append(
            {"xh": xh, "wall": wallh, "bias": bias, "biasr": biasr, "bch": bch}
        )
    return in_maps


def run(inputs: dict, trace: bool = False):
    """Run on all 8 cores; returns (full_output, BassKernelResults)."""
    nc = get_nc()
    in_maps = make_in_maps(**inputs)
    res = run_bass_kernel_spmd(
        nc, in_maps, core_ids=list(range(NCORES)), trace=trace
    )
    outs = []
    for r in res.results:
        o = np.asarray(r["out"]).astype(np.float32)  # [128, NT, VA]
        o = o[:, :, 0:DQ] / o[:, :, DQ:VA]  # host-side softmax normalization
        outs.append(o.transpose(1, 0, 2).reshape(RPC, DQ))
    out = np.concatenate(outs, axis=0)
    return out, res


def kernel(**inputs) -> np.ndarray:
    out, _ = run(inputs, trace=False)
    return out
